# revision 18
# baseline (speedup 1.0000x reference)
"""Self-attention (8 heads, d=64, B=2, N=4096, D=512) on 8 TRN2 NeuronCores.

Sharding: batch*heads across cores — core c handles batch b=c//4, heads
(2*(c%4), 2*(c%4)+1). Projection weights are sliced per-core on the host;
x is pre-transposed on the host so the device needs no transposes at all.

v2: software-pipelined attention loop with the softmax exp split across
BOTH the Scalar (ACT) and Vector (DVE) engines:
  - ACT computes exp(sc*SCALE) for one 512-wide half of each score tile
    (hardware spline, exact).
  - DVE computes the other half with a Schraudolph-style bit-trick:
    bf16_bits(e^x) ~= int16(x * 128*log2e*SCALE + 128*(127-0.0573)),
    emitted as one tensor_scalar (mult,add) with an int16-bitcast write
    into the bf16 attn tile (fp32->int16 conversion rounds-to-nearest).
  The halves alternate with kc parity so every query row mixes exact and
  approximated weights (rel err ~9e-3 vs 2e-2 budget).
Pipelined emission per kc: sc MMs (kc) -> exps (kc) -> av MMs (kc-1), so
the PE never idles waiting on the exp and the HAM clock-gate stays warm.

Device dataflow (per core, fully transposed "scoresT" formulation):
  qT2/kT2 [hd=128, n]  = W.T-chunks @ xT-chunks          (PE)
  v2      [n, hd+ones] natural                            (PE, bf16 store)
  per qq (1024 queries), kc (128 keys), h (2 heads):
    scT psum[128k,1024q] = kh.T @ qh   (interleaved h0/h1 -> row-group pairs)
    attnT = exp(scT*SCALE) -> bf16 SBUF   (ACT half | DVE half)
    av[65,1024] += v2'[kc].T @ attnT      (PE, accumulate; ones col = denom)
  drain av -> SBUF (ScalarE copy), reciprocal_approx_fast on denom row,
  DMA-broadcast, normalize mul -> outT (DVE)
  partial[n,512] = sum_h outT[h].T @ woT[h]               (PE)
Host: out[b] = sum of its 4 cores' partials + bo.
"""
import numpy as np
import ml_dtypes
from contextlib import ExitStack

import concourse.bass as bass
from concourse import bacc
import concourse.mybir as mybir
import concourse.tile as tile
from concourse.bass_utils import run_bass_kernel_spmd

B, N, D = 2, 4096, 512
HEADS, DH = 8, 64
SCALE = DH ** -0.5

F32 = mybir.dt.float32
F32R = mybir.dt.bfloat16  # matmul operand dtype (bf16: 1cyc/row)
BF16 = mybir.dt.bfloat16
I16 = mybir.dt.int16

QQ_W = 1024          # q-chunk width in the attention loop
N_QQ = N // QQ_W     # 4
N_KC = N // 128      # 32 key chunks
DCH = D // 128       # 4 contraction chunks for projections

LOG2E = 1.4426950408889634
A_SCH = float(128.0 * LOG2E * SCALE)          # fold attention scale in
B_SCH = float(128.0 * (127.0 - 0.057304959))  # equal-ripple bias

EXP_MODE = "whole"   # 'whole' = h0 tile on ACT, h1 tile on DVE Schraudolph;
                     # 'split' = each tile half ACT / half DVE; 'act' = all ACT
RECIP_MODE = "dma"   # 'dma' = exact reciprocal on a [128,8] DMA-reshaped view;
                     # 'exact' = nc.vector.reciprocal on [1,1024]


def build_bass():
    nc = bacc.Bacc(None, target_bir_lowering=False)

    xT = nc.dram_tensor("xT", [D, N], F32R, kind="ExternalInput")
    wqT = nc.dram_tensor("wqT", [D, 128], F32R, kind="ExternalInput")
    wkT = nc.dram_tensor("wkT", [D, 128], F32R, kind="ExternalInput")
    wvT = nc.dram_tensor("wvT", [D, 128], F32R, kind="ExternalInput")
    woT = nc.dram_tensor("woT", [2, 64, D], F32R, kind="ExternalInput")
    out = nc.dram_tensor("out", [N, D], F32, kind="ExternalOutput")
    recip_dram = nc.dram_tensor("recip_scratch", [N_QQ, 2, QQ_W], F32)
    denom_dram = nc.dram_tensor("denom_scratch", [N_QQ, 2, QQ_W], F32)

    with tile.TileContext(nc) as tc, ExitStack() as ctx:
        const = ctx.enter_context(tc.tile_pool(name="const", bufs=1))

        # ---- load inputs: tiny weights FIRST so the first proj matmul only
        # waits for them + the first xT chunk, not the whole 4MB of x ----
        wk_sb = const.tile([128, DCH, 128], F32R)
        nc.sync.dma_start(out=wk_sb, in_=wkT.rearrange("(c p) m -> p c m", p=128))
        wq_sb = const.tile([128, DCH, 128], F32R)
        nc.sync.dma_start(out=wq_sb, in_=wqT.rearrange("(c p) m -> p c m", p=128))
        wv_sb = const.tile([128, DCH, 128], F32R)
        nc.sync.dma_start(out=wv_sb, in_=wvT.rearrange("(c p) m -> p c m", p=128))
        wo_sb = const.tile([64, 2, D], F32R)
        nc.sync.dma_start(out=wo_sb, in_=woT.rearrange("h d n -> d h n"))
        xT_sb = const.tile([128, DCH, N], F32R)            # xT[(c p), n] -> [p, c, n]
        xT_r = xT.rearrange("(c p) n -> p c n", p=128)
        for nch in range(8):
            nc.sync.dma_start(out=xT_sb[:, :, bass.ts(nch, N // 8)],
                              in_=xT_r[:, :, bass.ts(nch, N // 8)])

        qT2 = const.tile([128, N], F32R)                   # [2-head d, n]
        kT2 = const.tile([128, N], F32R)
        v2 = const.tile([128, N_KC, 130], BF16)            # [k-part, kc, (v_h0|1|v_h1|1)]
        outT = const.tile([64, 2, N], F32R)                # normalized per-head av

        # ---- projections ----
        # q/k: c-outer over 8 psum accumulators so each weight chunk loads
        # once per 8 matmuls (weight changes 32 -> 4 per projection)
        with tc.tile_pool(name="qk_psum", bufs=8, space="PSUM") as qk_psum:
            for wi, (wsb, dst) in enumerate(((wk_sb, kT2), (wq_sb, qT2))):
                ps = [qk_psum.tile([128, 512], F32, tag="pj", name=f"pj_{wi}_{i}")
                      for i in range(8)]
                for c in range(DCH):
                    for nt in range(N // 512):
                        nc.tensor.matmul(ps[nt], wsb[:, c, :],
                                         xT_sb[:, c, bass.ts(nt, 512)],
                                         start=(c == 0), stop=(c == DCH - 1))
                for nt in range(N // 512):
                    nc.vector.tensor_copy(dst[:, bass.ts(nt, 512)], ps[nt])
        with tc.tile_pool(name="proj_psum", bufs=3, space="PSUM") as proj_psum:
            # v natural: out[n-tile, hd] = xT-chunk.T @ wv-chunk
            for kc in range(N_KC):
                pv = proj_psum.tile([128, 128], F32, tag="pv")
                for c in range(DCH):
                    nc.tensor.matmul(pv, xT_sb[:, c, bass.ts(kc, 128)], wv_sb[:, c, :],
                                     start=(c == 0), stop=(c == DCH - 1))
                # interleave the two heads' 64-col halves into v2 (cols 0-63, 65-128)
                nc.vector.tensor_copy(v2[:, kc, 0:64], pv[:, 0:64])
                nc.vector.tensor_copy(v2[:, kc, 65:129], pv[:, 64:128])
        # ones columns for the softmax-denominator trick
        nc.vector.memset(v2[:, :, 64], 1.0)
        nc.vector.memset(v2[:, :, 129], 1.0)

        # ---- attention (pipelined: sc(kc) | exp(kc) | av(kc-1)) ----
        with (
            tc.tile_pool(name="sc_psum", bufs=2, space="PSUM") as sc_psum,
            tc.tile_pool(name="av_psum", bufs=2, space="PSUM") as av_psum,
            tc.tile_pool(name="attn_sb", bufs=4) as attn_sb,
            tc.tile_pool(name="avs_pool", bufs=2) as avs_pool,
            tc.tile_pool(name="norm_sb", bufs=2) as norm_sb,
        ):
            # (EXP_MODE/RECIP_MODE now fixed: whole-tile exps, dma-reshape recip)
            # h-grouped emission: consecutive same-weight MMs skip the ~90ns
            # exposed LDWEIGHTS cost of a weight change
            def emit_sc(qq, kc, scs):
                for h in range(2):
                    for s in range(2):
                        nc.tensor.matmul(
                            scs[h][:, bass.ts(s, 512)],
                            kT2[h * 64:(h + 1) * 64, bass.ts(kc, 128)],
                            qT2[h * 64:(h + 1) * 64,
                                qq * QQ_W + s * 512:qq * QQ_W + (s + 1) * 512],
                            start=True, stop=True)

            def emit_exps(qq, kc, scs):
                ats = []
                for h in range(2):
                    at = attn_sb.tile([128, QQ_W], BF16, tag="at",
                                      name=f"at_{qq}_{kc}_{h}")
                    ats.append(at)
                    if h == 0:
                        nc.scalar.activation(
                            at, scs[h], mybir.ActivationFunctionType.Exp,
                            scale=float(SCALE))
                    else:
                        nc.vector.tensor_scalar(
                            at.bitcast(I16), scs[h], A_SCH, B_SCH,
                            mybir.AluOpType.mult, mybir.AluOpType.add)
                return ats

            def emit_av(avs, pkc, p_at):
                for h in range(2):
                    for s in range(2):
                        nc.tensor.matmul(
                            avs[h][:, bass.ts(s, 512)],
                            v2[:, pkc, h * 65:(h + 1) * 65],
                            p_at[h][:, bass.ts(s, 512)],
                            start=(pkc == 0), stop=(pkc == N_KC - 1))

            # deferred per-qq normalize work, spread one step per kc so the
            # PE never starves at qq boundaries (HAM stays warm)
            def norm_steps(qq, avs):
                avs_sb = avs_pool.tile([65, 2, QQ_W], F32, tag="avs",
                                       name=f"avs_{qq}")
                # step 0: drain h0 on ACT, h1 on DVE (frees av psum banks)
                yield lambda: (nc.scalar.copy(avs_sb[:, 0, :], avs[0]),
                               nc.vector.tensor_copy(avs_sb[:, 1, :], avs[1]))

                def recip(h):
                    nc.sync.dma_start(out=denom_dram[qq:qq + 1, h, :],
                                      in_=avs_sb[64:65, h, :])
                    dsrc = denom_dram[qq, h, :]
                    d2d = bass.AP(tensor=dsrc.tensor, offset=dsrc.offset,
                                  ap=[[8, 128], [1, 8]])
                    rin = norm_sb.tile([128, 8], F32, tag="rin",
                                       name=f"rin_{qq}_{h}")
                    nc.sync.dma_start(out=rin, in_=d2d)
                    rout = norm_sb.tile([128, 8], F32, tag="rout",
                                        name=f"rout_{qq}_{h}")
                    nc.vector.reciprocal(rout, rin)
                    rdst = recip_dram[qq, h, :]
                    r2d = bass.AP(tensor=rdst.tensor, offset=rdst.offset,
                                  ap=[[8, 128], [1, 8]])
                    nc.sync.dma_start(out=r2d, in_=rout)
                yield lambda: recip(0)
                yield lambda: recip(1)

                def bcast_mul(h):
                    bc = norm_sb.tile([64, QQ_W], F32, tag="bc",
                                      name=f"bc_{qq}_{h}")
                    src = recip_dram[qq, h, :]
                    bcast = bass.AP(tensor=src.tensor, offset=src.offset,
                                    ap=[[0, 64]] + src.ap)
                    nc.sync.dma_start(out=bc, in_=bcast)
                    nc.vector.tensor_mul(outT[:, h, qq * QQ_W:(qq + 1) * QQ_W],
                                         avs_sb[0:64, h, :], bc)
                yield lambda: bcast_mul(0)
                yield lambda: bcast_mul(1)

            prev_at = None      # at tiles of the previous (qq, kc)
            prev_avs = None     # av accumulators of the previous kc's qq
            pending = []        # deferred normalize steps from the last qq
            for qq in range(N_QQ):
                avs = [av_psum.tile([65, QQ_W], F32, tag="av", name=f"av_{qq}_{h}")
                       for h in range(2)]
                for kc in range(N_KC):
                    scs = [sc_psum.tile([128, QQ_W], F32, tag="sc",
                                        name=f"sc_{qq}_{kc}_{h}") for h in range(2)]
                    emit_sc(qq, kc, scs)
                    ats = emit_exps(qq, kc, scs)
                    if prev_at is not None:
                        emit_av(prev_avs, (kc - 1) % N_KC, prev_at)
                    if pending:
                        pending.pop(0)()
                    prev_at, prev_avs = ats, avs
                pending = list(norm_steps(qq, avs))
            # tail: trailing av of the last kc, then the last qq's normalize
            emit_av(prev_avs, N_KC - 1, prev_at)
            for step in pending:
                step()

        # ---- output projection ----
        with (
            tc.tile_pool(name="op_psum", bufs=3, space="PSUM") as op_psum,
            tc.tile_pool(name="op_sb", bufs=3) as op_sb,
        ):
            # earlier qq's outT is long finished; do the last qq's tiles last
            # so its normalize DMA chain hides behind the other 24 tiles
            nts = [nt for nt in range(N // 128) if nt < (N_QQ - 1) * (QQ_W // 128)]
            nts += [nt for nt in range(N // 128) if nt >= (N_QQ - 1) * (QQ_W // 128)]
            for nt in nts:
                po = op_psum.tile([128, D], F32, tag="po")
                nc.tensor.matmul(po, outT[:, 0, bass.ts(nt, 128)], wo_sb[:, 0, :],
                                 start=True, stop=False)
                nc.tensor.matmul(po, outT[:, 1, bass.ts(nt, 128)], wo_sb[:, 1, :],
                                 start=False, stop=True)
                ob = op_sb.tile([128, D], F32, tag="ob")
                nc.vector.tensor_copy(ob, po)
                nc.sync.dma_start(out=out[bass.ts(nt, 128), :], in_=ob)

    nc.compile()
    return nc


_NC_CACHE = None


def build_in_maps(x, Wq, Wk, Wv, Wo):
    bf = ml_dtypes.bfloat16
    x = np.asarray(x, np.float32)
    Wq, Wk, Wv, Wo = (np.asarray(a, np.float32) for a in (Wq, Wk, Wv, Wo))
    in_maps = []
    for c in range(8):
        b = c // 4
        h0 = 2 * (c % 4)
        xT = np.ascontiguousarray(x[b].T.astype(bf))
        wqT = np.ascontiguousarray(Wq[h0 * 64:(h0 + 2) * 64].T.astype(bf))
        wkT = np.ascontiguousarray(Wk[h0 * 64:(h0 + 2) * 64].T.astype(bf))
        wvT = np.ascontiguousarray(Wv[h0 * 64:(h0 + 2) * 64].T.astype(bf))
        woT = np.stack([np.ascontiguousarray(Wo[:, (h0 + h) * 64:(h0 + h + 1) * 64].T.astype(bf))
                        for h in range(2)])
        in_maps.append({"xT": xT, "wqT": wqT, "wkT": wkT, "wvT": wvT, "woT": woT})
    return in_maps


def kernel(x, Wq, Wk, Wv, Wo, bo):
    global _NC_CACHE
    bo = np.asarray(bo, np.float32)
    in_maps = build_in_maps(x, Wq, Wk, Wv, Wo)

    if _NC_CACHE is None:
        _NC_CACHE = build_bass()
    res = run_bass_kernel_spmd(_NC_CACHE, in_maps, list(range(8)))
    partials = [np.asarray(res.results[c]["out"], np.float32) for c in range(8)]

    out = np.empty((B, N, D), np.float32)
    for b in range(B):
        out[b] = partials[4 * b] + partials[4 * b + 1] + partials[4 * b + 2] + partials[4 * b + 3] + bo
    return out


if __name__ == "__main__":
    nc = build_bass()
    print("built ok")
